# revision 7
# baseline (speedup 1.0000x reference)
"""Pointer-attention kernel for one TRN2 chip (8 NeuronCores).

Decomposition (sequence-parallel, S=16384 sharded 8 ways):
  uh   = enc_hs @ W_ctx.T                       -> per-core [2048, 2048] matmul (fp32r)
  h    = relu(W_p1u @ tanh(uh) + c + b_p1)      with c = W_p1w @ tanh(wq) + W_p1d @ tanh(dh)
  sc   = W_p2 @ h  (masked)                     -> block-local softmax stats (flash style)
  ctx  = sum_s attn[s] * enc_hs[s]              -> per-block partials on VectorE
Host combines the 8 cores' (scores, max, sumexp, ctx-partial) into attn / ctx.
All big operands are host-transposed so the contraction axis lands on SBUF
partitions; matmuls run as fp32r (full speed at free-dim >= 256 on TRN2).
Row->column reshapes go through small DRAM roundtrips (PE transpose-mode and
gpsimd library ops are not usable on this runtime); partition-broadcast is a
K=1 matmul against a ones row.
"""

import sys

if "/opt/trn_rl_repo" not in sys.path:
    sys.path.insert(0, "/opt/trn_rl_repo")

from contextlib import ExitStack

import numpy as np

import concourse.bass as bass
import concourse.mybir as mybir
import concourse.tile as tile
from concourse import bacc
from concourse.bass_utils import run_bass_kernel_spmd

D = 2048
S = 16384
H = 250
NCORES = 8
SS = S // NCORES          # 2048 sequence positions per core
BLK = 256                 # s-block size
NBLK = SS // BLK          # 8 blocks per core
JPAD = 256                # projection dim padded 250 -> 256
NEG = -1.0e9              # masked-score fill; exp underflows to exactly 0

F32 = mybir.dt.float32
F32R = mybir.dt.float32r
I32 = mybir.dt.int32
AX = mybir.AxisListType
ALU = mybir.AluOpType
ACT = mybir.ActivationFunctionType


def build():
    nc = bacc.Bacc("TRN2", target_bir_lowering=False, debug=False, num_devices=NCORES)

    encT = nc.declare_dram_parameter("encT", [D, SS], F32R, isOutput=False)
    wctxT = nc.declare_dram_parameter("wctxT", [D, D], F32R, isOutput=False)
    winfoT = nc.declare_dram_parameter("winfoT", [D, D], F32R, isOutput=False)
    wqT = nc.declare_dram_parameter("wqT", [D, D], F32R, isOutput=False)
    w1uT = nc.declare_dram_parameter("w1uT", [D, JPAD], F32R, isOutput=False)
    w1wT = nc.declare_dram_parameter("w1wT", [D, JPAD], F32R, isOutput=False)
    w1dT = nc.declare_dram_parameter("w1dT", [D, JPAD], F32R, isOutput=False)
    w2T = nc.declare_dram_parameter("w2T", [JPAD, 1], F32R, isOutput=False)
    vcur = nc.declare_dram_parameter("vcur", [D, 1], F32R, isOutput=False)
    vsp = nc.declare_dram_parameter("vsp", [D, 1], F32R, isOutput=False)
    ones_d = nc.declare_dram_parameter("ones", [1, 128], F32R, isOutput=False)
    bq = nc.declare_dram_parameter("bq", [1, D], F32, isOutput=False)
    b1 = nc.declare_dram_parameter("b1", [1, JPAD], F32, isOutput=False)
    maski = nc.declare_dram_parameter("maski", [1, SS], I32, isOutput=False)

    oscores = nc.declare_dram_parameter("oscores", [1, SS], F32, isOutput=True)
    octx = nc.declare_dram_parameter("octx", [D, 1], F32, isOutput=True)
    ostats = nc.declare_dram_parameter("ostats", [1, 2], F32, isOutput=True)

    # DRAM scratch for row -> column reshapes
    tscr_wq = nc.dram_tensor("tscr_wq", [1, D], F32R)
    tscr_dh = nc.dram_tensor("tscr_dh", [1, D], F32R)
    tscr_cb = nc.dram_tensor("tscr_cb", [1, JPAD], F32)

    ET = D // 128   # 16 e-tiles (contraction over embedding)
    DT = D // 128   # 16 d-tiles

    with tile.TileContext(nc) as tc, ExitStack() as ctx:
        singles = ctx.enter_context(tc.tile_pool(name="singles", bufs=1))
        encp = ctx.enter_context(tc.tile_pool(name="encp", bufs=25))
        thp = ctx.enter_context(tc.tile_pool(name="thp", bufs=3))
        relup = ctx.enter_context(tc.tile_pool(name="relup", bufs=3))
        wstream = ctx.enter_context(tc.tile_pool(name="wstream", bufs=2))
        pbcp = ctx.enter_context(tc.tile_pool(name="pbcp", bufs=2))
        scrp = ctx.enter_context(tc.tile_pool(name="scrp", bufs=2))
        rowp = ctx.enter_context(tc.tile_pool(name="rowp", bufs=2))
        psA = ctx.enter_context(tc.tile_pool(name="psA", bufs=2, space="PSUM"))
        psB = ctx.enter_context(tc.tile_pool(name="psB", bufs=2, space="PSUM"))
        psC = ctx.enter_context(tc.tile_pool(name="psC", bufs=2, space="PSUM"))

        # ---- resident constants / accumulators ----
        wctx_res = singles.tile([128, ET, D], F32R, tag="wctx_res")
        w1u_res = singles.tile([128, DT, JPAD], F32R, tag="w1u_res")
        vcur_sb = singles.tile([128, ET], F32R, tag="vcur_sb")
        vsp_sb = singles.tile([128, ET], F32R, tag="vsp_sb")
        w2_sb = singles.tile([128, 2], F32R, tag="w2_sb")
        ones_sb = singles.tile([1, 128], F32R, tag="ones_sb")
        negrow = singles.tile([1, BLK], F32, tag="negrow")
        tcol_wq = singles.tile([128, ET], F32R, tag="tcol_wq")
        tcol_dh = singles.tile([128, ET], F32R, tag="tcol_dh")
        cb_cols = singles.tile([128, 2], F32, tag="cb_cols")
        cb_row = singles.tile([1, JPAD], F32, tag="cb_row")
        b1_sb = singles.tile([1, JPAD], F32, tag="b1_sb")
        mask_sb = singles.tile([1, SS], I32, tag="mask_sb")
        mrow = singles.tile([1, NBLK], F32, tag="mrow")
        lrow = singles.tile([1, NBLK], F32, tag="lrow")
        negm = singles.tile([1, 1], F32, tag="negm")
        wrow = singles.tile([1, NBLK], F32R, tag="wrow")
        st_sb = singles.tile([1, 2], F32, tag="st_sb")
        ctxp = [
            singles.tile([128, NBLK], F32, tag=f"ctxp{e}", name=f"ctxp{e}")
            for e in range(ET)
        ]
        ctxcol = singles.tile([128, 1], F32, tag="ctxcol")

        nc.vector.memset(negrow, NEG)

        # vectors arrive as [D,1]; view as [p, t] with p the partition axis
        nc.sync.dma_start(out=vcur_sb, in_=vcur.ap().rearrange("(t p) o -> p (t o)", p=128))
        nc.sync.dma_start(out=vsp_sb, in_=vsp.ap().rearrange("(t p) o -> p (t o)", p=128))
        nc.sync.dma_start(out=w2_sb, in_=w2T.ap().rearrange("(t p) o -> p (t o)", p=128))
        nc.sync.dma_start(out=ones_sb, in_=ones_d[:, :])
        nc.sync.dma_start(out=b1_sb, in_=b1[:, :])
        nc.sync.dma_start(out=mask_sb, in_=maski[:, :])

        # ---- GEMV phase: dh = W_info @ cur, wq = W_q @ s_prev + b_q ----
        for wmatT, vsb, tcol, tscr, with_bias in [
            (wqT, vsp_sb, tcol_wq, tscr_wq, True),
            (winfoT, vcur_sb, tcol_dh, tscr_dh, False),
        ]:
            for dg in range(D // 512):
                ps_row = psA.tile([1, 512], F32, tag="psA", name="ps_row")
                for e in range(ET):
                    wt = wstream.tile([128, 512], F32R, tag="wstream", name="wt")
                    nc.sync.dma_start(
                        out=wt, in_=wmatT[e * 128 : (e + 1) * 128, dg * 512 : (dg + 1) * 512]
                    )
                    nc.tensor.matmul(
                        ps_row, vsb[:, e : e + 1], wt,
                        start=(e == 0), stop=(e == ET - 1),
                    )
                throw = rowp.tile([1, 512], F32R, tag="throw", name="throw")
                if with_bias:
                    bqt = rowp.tile([1, 512], F32, tag="bqt", name="bqt")
                    nc.sync.dma_start(out=bqt, in_=bq[:, dg * 512 : (dg + 1) * 512])
                    nc.vector.tensor_add(throw, ps_row, bqt)
                    nc.scalar.activation(out=throw, in_=throw, func=ACT.Tanh)
                else:
                    nc.scalar.activation(out=throw, in_=ps_row, func=ACT.Tanh)
                nc.sync.dma_start(out=tscr[:, dg * 512 : (dg + 1) * 512], in_=throw)
            # tanh row -> [128, 16] column layout via DRAM roundtrip
            nc.sync.dma_start(
                out=tcol, in_=tscr.ap().rearrange("a (t p) -> p (t a)", p=128)
            )

        # ---- c = W1w @ tanh(wq) + W1d @ tanh(dh)  (+ b1) ----
        ps_c = psC.tile([1, JPAD], F32, tag="psC", name="ps_c")
        n_c_mm = 2 * DT
        i_mm = 0
        for tcol, wmat in [(tcol_wq, w1wT), (tcol_dh, w1dT)]:
            for t in range(DT):
                wt = wstream.tile([128, JPAD], F32R, tag="wstream", name="wtc")
                nc.sync.dma_start(out=wt, in_=wmat[t * 128 : (t + 1) * 128, :])
                nc.tensor.matmul(
                    ps_c, tcol[:, t : t + 1], wt,
                    start=(i_mm == 0), stop=(i_mm == n_c_mm - 1),
                )
                i_mm += 1
        nc.vector.tensor_add(cb_row, ps_c, b1_sb)
        nc.sync.dma_start(out=tscr_cb[:, :], in_=cb_row)
        nc.sync.dma_start(
            out=cb_cols, in_=tscr_cb.ap().rearrange("a (t p) -> p (t a)", p=128)
        )

        # ---- resident weights for the big matmuls ----
        wctx_view = wctxT.ap().rearrange("(te p) d -> p te d", p=128)
        for te in range(ET):
            for dq in range(4):
                nc.sync.dma_start(
                    out=wctx_res[:, te, dq * 512 : (dq + 1) * 512],
                    in_=wctx_view[:, te, dq * 512 : (dq + 1) * 512],
                )
        w1u_view = w1uT.ap().rearrange("(td p) j -> p td j", p=128)
        for td in range(DT):
            nc.sync.dma_start(out=w1u_res[:, td, :], in_=w1u_view[:, td, :])

        # ---- main blocks ----
        for b in range(NBLK):
            enc_sb = []
            for e in range(ET):
                enc_t = encp.tile([128, BLK], F32R, tag="enc_t", name="enc_t")
                nc.sync.dma_start(
                    out=enc_t, in_=encT[e * 128 : (e + 1) * 128, b * BLK : (b + 1) * BLK]
                )
                enc_sb.append(enc_t)

            ps_h = [
                psB.tile([128, BLK], F32, tag="psB", name=f"ps_h{jt}")
                for jt in range(2)
            ]
            for td in range(DT):
                ps_uh = psA.tile([128, BLK], F32, tag="psA", name="ps_uh")
                for e in range(ET):
                    nc.tensor.matmul(
                        ps_uh,
                        wctx_res[:, e, td * 128 : (td + 1) * 128],
                        enc_sb[e],
                        start=(e == 0), stop=(e == ET - 1),
                    )
                th = thp.tile([128, BLK], F32R, tag="th", name="th")
                nc.scalar.activation(out=th, in_=ps_uh, func=ACT.Tanh)
                for jt in range(2):
                    nc.tensor.matmul(
                        ps_h[jt],
                        w1u_res[:, td, jt * 128 : (jt + 1) * 128],
                        th,
                        start=(td == 0), stop=(td == DT - 1),
                    )

            ps_s = psC.tile([1, BLK], F32, tag="psC", name="ps_s")
            for jt in range(2):
                reluh = relup.tile([128, BLK], F32R, tag="reluh", name="reluh")
                nc.scalar.activation(
                    out=reluh, in_=ps_h[jt], func=ACT.Relu,
                    bias=cb_cols[:, jt : jt + 1], scale=1.0,
                )
                nc.tensor.matmul(
                    ps_s, w2_sb[:, jt : jt + 1], reluh,
                    start=(jt == 0), stop=(jt == 1),
                )

            srow = rowp.tile([1, BLK], F32, tag="srow", name="srow")
            nc.vector.select(
                srow, mask_sb[:, b * BLK : (b + 1) * BLK], negrow, ps_s
            )
            nc.sync.dma_start(out=oscores[:, b * BLK : (b + 1) * BLK], in_=srow)
            nc.vector.tensor_reduce(
                out=mrow[:, b : b + 1], in_=srow, axis=AX.X, op=ALU.max
            )
            nc.vector.tensor_scalar_mul(negm, mrow[:, b : b + 1], -1.0)
            prow = rowp.tile([1, BLK], F32R, tag="prow", name="prow")
            nc.scalar.activation(
                out=prow, in_=srow, func=ACT.Exp,
                bias=negm[0:1, 0:1], scale=1.0, accum_out=lrow[:, b : b + 1],
            )
            # broadcast exp row to 128 partitions: ones[1,128].T @ prow[1,BLK]
            ps_bc = psA.tile([128, BLK], F32, tag="psA", name="ps_bc")
            nc.tensor.matmul(ps_bc, ones_sb, prow, start=True, stop=True)
            pbc = pbcp.tile([128, BLK], F32, tag="pbc", name="pbc")
            nc.vector.tensor_copy(pbc, ps_bc)
            for e in range(ET):
                scr = scrp.tile([128, BLK], F32, tag="scr", name="scr")
                nc.vector.tensor_mul(scr, enc_sb[e].bitcast(F32), pbc)
                nc.vector.tensor_reduce(
                    out=ctxp[e][:, b : b + 1], in_=scr, axis=AX.X, op=ALU.add
                )

        # ---- per-core combine ----
        nc.vector.tensor_reduce(out=st_sb[:, 0:1], in_=mrow, axis=AX.X, op=ALU.max)
        nc.vector.tensor_scalar_mul(negm, st_sb[:, 0:1], -1.0)
        nc.scalar.activation(
            out=wrow, in_=mrow, func=ACT.Exp, bias=negm[0:1, 0:1], scale=1.0
        )
        ljunk = rowp.tile([1, NBLK], F32, tag="ljunk", name="ljunk")
        nc.vector.tensor_mul(ljunk, lrow, wrow.bitcast(F32))
        nc.vector.tensor_reduce(
            out=st_sb[:, 1:2], in_=ljunk, axis=AX.X, op=ALU.add
        )
        nc.sync.dma_start(out=ostats[:, :], in_=st_sb)

        ps_wb = psA.tile([128, NBLK], F32, tag="psA", name="ps_wb")
        nc.tensor.matmul(ps_wb, ones_sb, wrow, start=True, stop=True)
        wbc = pbcp.tile([128, NBLK], F32, tag="pbc", name="wbc")
        nc.vector.tensor_copy(wbc, ps_wb)
        for e in range(ET):
            scr8 = scrp.tile([128, NBLK], F32, tag="scr", name="scr8")
            nc.vector.tensor_mul(scr8, ctxp[e], wbc)
            nc.vector.tensor_reduce(
                out=ctxcol[:, 0:1], in_=scr8, axis=AX.X, op=ALU.add
            )
            nc.sync.dma_start(out=octx[e * 128 : (e + 1) * 128, 0:1], in_=ctxcol)

    nc.compile()
    return nc


_CACHE = {}


def _marshal(s_prev, enc_hs, cur_men_rep, src_mask, W_info, W_ctx, W_q, b_q,
             W_p1, b_p1, W_p2, b_p2):
    f = np.float32
    encT = np.ascontiguousarray(np.asarray(enc_hs, f)[0].T)            # [D, S]
    shared = {
        "wctxT": np.ascontiguousarray(np.asarray(W_ctx, f).T),
        "winfoT": np.ascontiguousarray(np.asarray(W_info, f).T),
        "wqT": np.ascontiguousarray(np.asarray(W_q, f).T),
        "vcur": np.asarray(cur_men_rep, f).reshape(D, 1).copy(),
        "vsp": np.asarray(s_prev, f).reshape(D, 1).copy(),
        "bq": np.asarray(b_q, f).reshape(1, D).copy(),
        "ones": np.ones((1, 128), f),
    }
    W1 = np.asarray(W_p1, f)
    for name, sl in [("w1uT", slice(0, D)), ("w1wT", slice(D, 2 * D)),
                     ("w1dT", slice(2 * D, 3 * D))]:
        t = np.zeros((D, JPAD), f)
        t[:, :H] = W1[:, sl].T
        shared[name] = t
    w2 = np.zeros((JPAD, 1), f)
    w2[:H, 0] = np.asarray(W_p2, f)[0]
    shared["w2T"] = w2
    b1pad = np.zeros((1, JPAD), f)
    b1pad[0, :H] = np.asarray(b_p1, f)
    shared["b1"] = b1pad
    mk = np.ascontiguousarray(np.asarray(src_mask).reshape(S).astype(np.int32))
    in_maps = []
    for c in range(NCORES):
        m = dict(shared)
        m["encT"] = np.ascontiguousarray(encT[:, c * SS : (c + 1) * SS])
        m["maski"] = mk[c * SS : (c + 1) * SS].reshape(1, SS).copy()
        in_maps.append(m)
    return in_maps


def kernel(**inputs):
    if "nc" not in _CACHE:
        _CACHE["nc"] = build()
    nc = _CACHE["nc"]
    in_maps = _marshal(**inputs)
    res = run_bass_kernel_spmd(nc, in_maps, list(range(NCORES)))
    scores = np.concatenate([res.results[c]["oscores"][0] for c in range(NCORES)])
    m_core = np.array([res.results[c]["ostats"][0, 0] for c in range(NCORES)])
    l_core = np.array([res.results[c]["ostats"][0, 1] for c in range(NCORES)])
    ctx_core = np.stack([res.results[c]["octx"][:, 0] for c in range(NCORES)])
    mg = m_core.max()
    wc = np.exp(m_core - mg)
    Z = float((wc * l_core).sum())
    attn = (np.exp(scores - mg) / Z).astype(np.float32)
    ctx = ((wc[:, None] * ctx_core).sum(0, keepdims=True) / Z).astype(np.float32)
    return attn, ctx


# revision 8
# speedup vs baseline: 1.0701x; 1.0701x over previous
"""Pointer-attention kernel for one TRN2 chip (8 NeuronCores).

Decomposition (sequence-parallel, S=16384 sharded 8 ways):
  uh   = enc_hs @ W_ctx.T                       -> per-core [2048, 2048] matmul (fp32r)
  h    = relu(W_p1u @ tanh(uh) + c + b_p1)      with c = W_p1w @ tanh(wq) + W_p1d @ tanh(dh)
  sc   = W_p2 @ h  (masked)                     -> block-local softmax stats (flash style)
  ctx  = sum_s attn[s] * enc_hs[s]              -> per-block partials on VectorE
Host combines the 8 cores' (scores, max, sumexp, ctx-partial) into attn / ctx.
All big operands are host-transposed so the contraction axis lands on SBUF
partitions; matmuls run as fp32r (full speed at free-dim >= 256 on TRN2).
Row->column reshapes go through small DRAM roundtrips (PE transpose-mode and
gpsimd library ops are not usable on this runtime); partition-broadcast is a
K=1 matmul against a ones row.
"""

import sys

if "/opt/trn_rl_repo" not in sys.path:
    sys.path.insert(0, "/opt/trn_rl_repo")

from contextlib import ExitStack

import numpy as np

import concourse.bass as bass
import concourse.mybir as mybir
import concourse.tile as tile
from concourse import bacc
from concourse.bass_utils import run_bass_kernel_spmd

D = 2048
S = 16384
H = 250
NCORES = 8
SS = S // NCORES          # 2048 sequence positions per core
BLK = 256                 # s-block size
NBLK = SS // BLK          # 8 blocks per core
JPAD = 256                # projection dim padded 250 -> 256
NEG = -1.0e9              # masked-score fill; exp underflows to exactly 0

F32 = mybir.dt.float32
F32R = mybir.dt.float32r
I32 = mybir.dt.int32
BF16 = mybir.dt.bfloat16
AX = mybir.AxisListType
ALU = mybir.AluOpType
ACT = mybir.ActivationFunctionType


def build():
    nc = bacc.Bacc("TRN2", target_bir_lowering=False, debug=False, num_devices=NCORES)

    encT = nc.declare_dram_parameter("encT", [D, SS], BF16, isOutput=False)
    wctxT = nc.declare_dram_parameter("wctxT", [D, D], BF16, isOutput=False)
    winfoT = nc.declare_dram_parameter("winfoT", [D, D], F32R, isOutput=False)
    wqT = nc.declare_dram_parameter("wqT", [D, D], F32R, isOutput=False)
    w1uT = nc.declare_dram_parameter("w1uT", [D, JPAD], BF16, isOutput=False)
    w1wT = nc.declare_dram_parameter("w1wT", [D, JPAD], F32R, isOutput=False)
    w1dT = nc.declare_dram_parameter("w1dT", [D, JPAD], F32R, isOutput=False)
    w2T = nc.declare_dram_parameter("w2T", [JPAD, 1], BF16, isOutput=False)
    vcur = nc.declare_dram_parameter("vcur", [D, 1], F32R, isOutput=False)
    vsp = nc.declare_dram_parameter("vsp", [D, 1], F32R, isOutput=False)
    ones_d = nc.declare_dram_parameter("ones", [1, 128], F32R, isOutput=False)
    bq = nc.declare_dram_parameter("bq", [1, D], F32, isOutput=False)
    b1 = nc.declare_dram_parameter("b1", [1, JPAD], F32, isOutput=False)
    maski = nc.declare_dram_parameter("maski", [1, SS], I32, isOutput=False)

    oscores = nc.declare_dram_parameter("oscores", [1, SS], F32, isOutput=True)
    octx = nc.declare_dram_parameter("octx", [D, 1], F32, isOutput=True)
    ostats = nc.declare_dram_parameter("ostats", [1, 2], F32, isOutput=True)

    # DRAM scratch for row -> column reshapes
    tscr_wq = nc.dram_tensor("tscr_wq", [1, D], F32R)
    tscr_dh = nc.dram_tensor("tscr_dh", [1, D], F32R)
    tscr_cb = nc.dram_tensor("tscr_cb", [1, JPAD], F32)

    ET = D // 128   # 16 e-tiles (contraction over embedding)
    DT = D // 128   # 16 d-tiles

    with tile.TileContext(nc) as tc, ExitStack() as ctx:
        singles = ctx.enter_context(tc.tile_pool(name="singles", bufs=1))
        encp = ctx.enter_context(tc.tile_pool(name="encp", bufs=40))
        thp = ctx.enter_context(tc.tile_pool(name="thp", bufs=4))
        relup = ctx.enter_context(tc.tile_pool(name="relup", bufs=3))
        wstream = ctx.enter_context(tc.tile_pool(name="wstream", bufs=4))
        pbcp = ctx.enter_context(tc.tile_pool(name="pbcp", bufs=2))
        scrp = ctx.enter_context(tc.tile_pool(name="scrp", bufs=2))
        rowp = ctx.enter_context(tc.tile_pool(name="rowp", bufs=3))
        psA = ctx.enter_context(tc.tile_pool(name="psA", bufs=2, space="PSUM"))
        psB = ctx.enter_context(tc.tile_pool(name="psB", bufs=2, space="PSUM"))
        psC = ctx.enter_context(tc.tile_pool(name="psC", bufs=2, space="PSUM"))

        # ---- resident constants / accumulators ----
        wctx_res = singles.tile([128, ET, D], BF16, tag="wctx_res")
        w1u_res = singles.tile([128, DT, JPAD], BF16, tag="w1u_res")
        vcur_sb = singles.tile([128, ET], F32R, tag="vcur_sb")
        vsp_sb = singles.tile([128, ET], F32R, tag="vsp_sb")
        w2_sb = singles.tile([128, 2], BF16, tag="w2_sb")
        ones_sb = singles.tile([1, 128], F32R, tag="ones_sb")
        negrow = singles.tile([1, BLK], F32, tag="negrow")
        tcol_wq = singles.tile([128, ET], F32R, tag="tcol_wq")
        tcol_dh = singles.tile([128, ET], F32R, tag="tcol_dh")
        cb_cols = singles.tile([128, 2], F32, tag="cb_cols")
        cb_row = singles.tile([1, JPAD], F32, tag="cb_row")
        b1_sb = singles.tile([1, JPAD], F32, tag="b1_sb")
        mask_sb = singles.tile([1, SS], I32, tag="mask_sb")
        mrow = singles.tile([1, NBLK], F32, tag="mrow")
        lrow = singles.tile([1, NBLK], F32, tag="lrow")
        negm = singles.tile([1, 1], F32, tag="negm")
        wrow = singles.tile([1, NBLK], F32R, tag="wrow")
        st_sb = singles.tile([1, 2], F32, tag="st_sb")
        ctxp = [
            singles.tile([128, NBLK], F32, tag=f"ctxp{e}", name=f"ctxp{e}")
            for e in range(ET)
        ]
        ctxcol = singles.tile([128, 1], F32, tag="ctxcol")

        nc.vector.memset(negrow, NEG)

        # vectors arrive as [D,1]; view as [p, t] with p the partition axis
        nc.sync.dma_start(out=vcur_sb, in_=vcur.ap().rearrange("(t p) o -> p (t o)", p=128))
        nc.sync.dma_start(out=vsp_sb, in_=vsp.ap().rearrange("(t p) o -> p (t o)", p=128))
        nc.sync.dma_start(out=w2_sb, in_=w2T.ap().rearrange("(t p) o -> p (t o)", p=128))
        nc.sync.dma_start(out=ones_sb, in_=ones_d[:, :])
        nc.sync.dma_start(out=b1_sb, in_=b1[:, :])
        nc.sync.dma_start(out=mask_sb, in_=maski[:, :])

        # ---- GEMV phase: dh = W_info @ cur, wq = W_q @ s_prev + b_q ----
        for wmatT, vsb, tcol, tscr, with_bias in [
            (wqT, vsp_sb, tcol_wq, tscr_wq, True),
            (winfoT, vcur_sb, tcol_dh, tscr_dh, False),
        ]:
            for dg in range(D // 512):
                ps_row = psA.tile([1, 512], F32, tag="psA", name="ps_row")
                for e in range(ET):
                    wt = wstream.tile([128, 512], F32R, tag="wstream", name="wt")
                    nc.sync.dma_start(
                        out=wt, in_=wmatT[e * 128 : (e + 1) * 128, dg * 512 : (dg + 1) * 512]
                    )
                    nc.tensor.matmul(
                        ps_row, vsb[:, e : e + 1], wt,
                        start=(e == 0), stop=(e == ET - 1),
                    )
                throw = rowp.tile([1, 512], F32R, tag="throw", name="throw")
                if with_bias:
                    bqt = rowp.tile([1, 512], F32, tag="bqt", name="bqt")
                    nc.sync.dma_start(out=bqt, in_=bq[:, dg * 512 : (dg + 1) * 512])
                    nc.vector.tensor_add(throw, ps_row, bqt)
                    nc.scalar.activation(out=throw, in_=throw, func=ACT.Tanh)
                else:
                    nc.scalar.activation(out=throw, in_=ps_row, func=ACT.Tanh)
                nc.sync.dma_start(out=tscr[:, dg * 512 : (dg + 1) * 512], in_=throw)
            # tanh row -> [128, 16] column layout via DRAM roundtrip
            nc.sync.dma_start(
                out=tcol, in_=tscr.ap().rearrange("a (t p) -> p (t a)", p=128)
            )

        # ---- c = W1w @ tanh(wq) + W1d @ tanh(dh)  (+ b1) ----
        ps_c = psC.tile([1, JPAD], F32, tag="psC", name="ps_c")
        n_c_mm = 2 * DT
        i_mm = 0
        for tcol, wmat in [(tcol_wq, w1wT), (tcol_dh, w1dT)]:
            for t in range(DT):
                wt = wstream.tile([128, JPAD], F32R, tag="wstream", name="wtc")
                nc.sync.dma_start(out=wt, in_=wmat[t * 128 : (t + 1) * 128, :])
                nc.tensor.matmul(
                    ps_c, tcol[:, t : t + 1], wt,
                    start=(i_mm == 0), stop=(i_mm == n_c_mm - 1),
                )
                i_mm += 1
        nc.vector.tensor_add(cb_row, ps_c, b1_sb)
        nc.sync.dma_start(out=tscr_cb[:, :], in_=cb_row)
        nc.sync.dma_start(
            out=cb_cols, in_=tscr_cb.ap().rearrange("a (t p) -> p (t a)", p=128)
        )

        # ---- resident weights for the big matmuls ----
        wctx_view = wctxT.ap().rearrange("(te p) d -> p te d", p=128)
        for te in range(ET):
            for dq in range(4):
                nc.sync.dma_start(
                    out=wctx_res[:, te, dq * 512 : (dq + 1) * 512],
                    in_=wctx_view[:, te, dq * 512 : (dq + 1) * 512],
                )
        w1u_view = w1uT.ap().rearrange("(td p) j -> p td j", p=128)
        for td in range(DT):
            nc.sync.dma_start(out=w1u_res[:, td, :], in_=w1u_view[:, td, :])

        # ---- main blocks ----
        for b in range(NBLK):
            enc_sb = []
            for e in range(ET):
                enc_t = encp.tile([128, BLK], BF16, tag="enc_t", name="enc_t")
                nc.sync.dma_start(
                    out=enc_t, in_=encT[e * 128 : (e + 1) * 128, b * BLK : (b + 1) * BLK]
                )
                enc_sb.append(enc_t)

            ps_h = [
                psB.tile([128, BLK], F32, tag="psB", name=f"ps_h{jt}")
                for jt in range(2)
            ]
            for td in range(DT):
                ps_uh = psA.tile([128, BLK], F32, tag="psA", name="ps_uh")
                for e in range(ET):
                    nc.tensor.matmul(
                        ps_uh,
                        wctx_res[:, e, td * 128 : (td + 1) * 128],
                        enc_sb[e],
                        start=(e == 0), stop=(e == ET - 1),
                    )
                th = thp.tile([128, BLK], BF16, tag="th", name="th")
                nc.scalar.activation(out=th, in_=ps_uh, func=ACT.Tanh)
                for jt in range(2):
                    nc.tensor.matmul(
                        ps_h[jt],
                        w1u_res[:, td, jt * 128 : (jt + 1) * 128],
                        th,
                        start=(td == 0), stop=(td == DT - 1),
                    )

            ps_s = psC.tile([1, BLK], F32, tag="psC", name="ps_s")
            for jt in range(2):
                reluh = relup.tile([128, BLK], BF16, tag="reluh", name="reluh")
                nc.scalar.activation(
                    out=reluh, in_=ps_h[jt], func=ACT.Relu,
                    bias=cb_cols[:, jt : jt + 1], scale=1.0,
                )
                nc.tensor.matmul(
                    ps_s, w2_sb[:, jt : jt + 1], reluh,
                    start=(jt == 0), stop=(jt == 1),
                )

            srow = rowp.tile([1, BLK], F32, tag="srow", name="srow")
            nc.vector.select(
                srow, mask_sb[:, b * BLK : (b + 1) * BLK], negrow, ps_s
            )
            nc.sync.dma_start(out=oscores[:, b * BLK : (b + 1) * BLK], in_=srow)
            nc.vector.tensor_reduce(
                out=mrow[:, b : b + 1], in_=srow, axis=AX.X, op=ALU.max
            )
            nc.vector.tensor_scalar_mul(negm, mrow[:, b : b + 1], -1.0)
            prow = rowp.tile([1, BLK], F32R, tag="prow", name="prow")
            nc.scalar.activation(
                out=prow, in_=srow, func=ACT.Exp,
                bias=negm[0:1, 0:1], scale=1.0, accum_out=lrow[:, b : b + 1],
            )
            # broadcast exp row to 128 partitions: ones[1,128].T @ prow[1,BLK]
            ps_bc = psA.tile([128, BLK], F32, tag="psA", name="ps_bc")
            nc.tensor.matmul(ps_bc, ones_sb, prow, start=True, stop=True)
            pbc = pbcp.tile([128, BLK], F32, tag="pbc", name="pbc")
            nc.vector.tensor_copy(pbc, ps_bc)
            for e in range(ET):
                scr = scrp.tile([128, BLK], F32, tag="scr", name="scr")
                nc.vector.tensor_mul(scr, enc_sb[e], pbc)
                nc.vector.tensor_reduce(
                    out=ctxp[e][:, b : b + 1], in_=scr, axis=AX.X, op=ALU.add
                )

        # ---- per-core combine ----
        nc.vector.tensor_reduce(out=st_sb[:, 0:1], in_=mrow, axis=AX.X, op=ALU.max)
        nc.vector.tensor_scalar_mul(negm, st_sb[:, 0:1], -1.0)
        nc.scalar.activation(
            out=wrow, in_=mrow, func=ACT.Exp, bias=negm[0:1, 0:1], scale=1.0
        )
        ljunk = rowp.tile([1, NBLK], F32, tag="ljunk", name="ljunk")
        nc.vector.tensor_mul(ljunk, lrow, wrow.bitcast(F32))
        nc.vector.tensor_reduce(
            out=st_sb[:, 1:2], in_=ljunk, axis=AX.X, op=ALU.add
        )
        nc.sync.dma_start(out=ostats[:, :], in_=st_sb)

        ps_wb = psA.tile([128, NBLK], F32, tag="psA", name="ps_wb")
        nc.tensor.matmul(ps_wb, ones_sb, wrow, start=True, stop=True)
        wbc = pbcp.tile([128, NBLK], F32, tag="pbc", name="wbc")
        nc.vector.tensor_copy(wbc, ps_wb)
        for e in range(ET):
            scr8 = scrp.tile([128, NBLK], F32, tag="scr", name="scr8")
            nc.vector.tensor_mul(scr8, ctxp[e], wbc)
            nc.vector.tensor_reduce(
                out=ctxcol[:, 0:1], in_=scr8, axis=AX.X, op=ALU.add
            )
            nc.sync.dma_start(out=octx[e * 128 : (e + 1) * 128, 0:1], in_=ctxcol)

    nc.compile()
    return nc


_CACHE = {}


def _marshal(s_prev, enc_hs, cur_men_rep, src_mask, W_info, W_ctx, W_q, b_q,
             W_p1, b_p1, W_p2, b_p2):
    f = np.float32
    import ml_dtypes
    bf = ml_dtypes.bfloat16
    encT = np.ascontiguousarray(np.asarray(enc_hs, f)[0].T).astype(bf)  # [D, S]
    shared = {
        "wctxT": np.ascontiguousarray(np.asarray(W_ctx, f).T).astype(bf),
        "winfoT": np.ascontiguousarray(np.asarray(W_info, f).T),
        "wqT": np.ascontiguousarray(np.asarray(W_q, f).T),
        "vcur": np.asarray(cur_men_rep, f).reshape(D, 1).copy(),
        "vsp": np.asarray(s_prev, f).reshape(D, 1).copy(),
        "bq": np.asarray(b_q, f).reshape(1, D).copy(),
        "ones": np.ones((1, 128), f),
    }
    W1 = np.asarray(W_p1, f)
    for name, sl in [("w1uT", slice(0, D)), ("w1wT", slice(D, 2 * D)),
                     ("w1dT", slice(2 * D, 3 * D))]:
        t = np.zeros((D, JPAD), f)
        t[:, :H] = W1[:, sl].T
        shared[name] = t.astype(bf) if name == "w1uT" else t
    w2 = np.zeros((JPAD, 1), f)
    w2[:H, 0] = np.asarray(W_p2, f)[0]
    shared["w2T"] = w2.astype(bf)
    b1pad = np.zeros((1, JPAD), f)
    b1pad[0, :H] = np.asarray(b_p1, f)
    shared["b1"] = b1pad
    mk = np.ascontiguousarray(np.asarray(src_mask).reshape(S).astype(np.int32))
    in_maps = []
    for c in range(NCORES):
        m = dict(shared)
        m["encT"] = np.ascontiguousarray(encT[:, c * SS : (c + 1) * SS])
        m["maski"] = mk[c * SS : (c + 1) * SS].reshape(1, SS).copy()
        in_maps.append(m)
    return in_maps


def kernel(**inputs):
    if "nc" not in _CACHE:
        _CACHE["nc"] = build()
    nc = _CACHE["nc"]
    in_maps = _marshal(**inputs)
    res = run_bass_kernel_spmd(nc, in_maps, list(range(NCORES)))
    scores = np.concatenate([res.results[c]["oscores"][0] for c in range(NCORES)])
    m_core = np.array([res.results[c]["ostats"][0, 0] for c in range(NCORES)])
    l_core = np.array([res.results[c]["ostats"][0, 1] for c in range(NCORES)])
    ctx_core = np.stack([res.results[c]["octx"][:, 0] for c in range(NCORES)])
    mg = m_core.max()
    wc = np.exp(m_core - mg)
    Z = float((wc * l_core).sum())
    attn = (np.exp(scores - mg) / Z).astype(np.float32)
    ctx = ((wc[:, None] * ctx_core).sum(0, keepdims=True) / Z).astype(np.float32)
    return attn, ctx


# revision 13
# speedup vs baseline: 1.0820x; 1.0111x over previous
"""Pointer-attention kernel for one TRN2 chip (8 NeuronCores).

Decomposition (sequence-parallel, S=16384 sharded 8 ways):
  uh   = enc_hs @ W_ctx.T                       -> per-core [2048, 2048] matmul (fp32r)
  h    = relu(W_p1u @ tanh(uh) + c + b_p1)      with c = W_p1w @ tanh(wq) + W_p1d @ tanh(dh)
  sc   = W_p2 @ h  (masked)                     -> block-local softmax stats (flash style)
  ctx  = sum_s attn[s] * enc_hs[s]              -> per-block partials on VectorE
Host combines the 8 cores' (scores, max, sumexp, ctx-partial) into attn / ctx.
All big operands are host-transposed so the contraction axis lands on SBUF
partitions; matmuls run as fp32r (full speed at free-dim >= 256 on TRN2).
Row->column reshapes go through small DRAM roundtrips (PE transpose-mode and
gpsimd library ops are not usable on this runtime); partition-broadcast is a
K=1 matmul against a ones row.
"""

import sys

if "/opt/trn_rl_repo" not in sys.path:
    sys.path.insert(0, "/opt/trn_rl_repo")

from contextlib import ExitStack

import numpy as np

import concourse.bass as bass
import concourse.mybir as mybir
import concourse.tile as tile
from concourse import bacc
from concourse.bass_utils import run_bass_kernel_spmd

D = 2048
S = 16384
H = 250
NCORES = 8
SS = S // NCORES          # 2048 sequence positions per core
BLK = 512                 # s-block size (one full PSUM bank)
NBLK = SS // BLK          # 4 blocks per core
JPAD = 256                # projection dim padded 250 -> 256
NEG = -1.0e9              # masked-score fill; exp underflows to exactly 0

F32 = mybir.dt.float32
F32R = mybir.dt.float32r
I32 = mybir.dt.int32
BF16 = mybir.dt.bfloat16
AX = mybir.AxisListType
ALU = mybir.AluOpType
ACT = mybir.ActivationFunctionType


def build():
    nc = bacc.Bacc("TRN2", target_bir_lowering=False, debug=False, num_devices=NCORES)

    encT = nc.declare_dram_parameter("encT", [D, SS], BF16, isOutput=False)
    wctxT = nc.declare_dram_parameter("wctxT", [D, D], BF16, isOutput=False)
    winfoT = nc.declare_dram_parameter("winfoT", [D, D], F32R, isOutput=False)
    wqT = nc.declare_dram_parameter("wqT", [D, D], F32R, isOutput=False)
    w1uT = nc.declare_dram_parameter("w1uT", [D, JPAD], BF16, isOutput=False)
    w1wT = nc.declare_dram_parameter("w1wT", [D, JPAD], F32R, isOutput=False)
    w1dT = nc.declare_dram_parameter("w1dT", [D, JPAD], F32R, isOutput=False)
    w2T = nc.declare_dram_parameter("w2T", [JPAD, 1], BF16, isOutput=False)
    vcur = nc.declare_dram_parameter("vcur", [D, 1], F32R, isOutput=False)
    vsp = nc.declare_dram_parameter("vsp", [D, 1], F32R, isOutput=False)
    ones_d = nc.declare_dram_parameter("ones", [1, 128], F32R, isOutput=False)
    bq = nc.declare_dram_parameter("bq", [1, D], F32, isOutput=False)
    b1 = nc.declare_dram_parameter("b1", [1, JPAD], F32, isOutput=False)
    maski = nc.declare_dram_parameter("maski", [1, SS], I32, isOutput=False)

    oscores = nc.declare_dram_parameter("oscores", [1, SS], F32, isOutput=True)
    octx = nc.declare_dram_parameter("octx", [D, 1], F32, isOutput=True)
    ostats = nc.declare_dram_parameter("ostats", [1, 2], F32, isOutput=True)

    # DRAM scratch for row -> column reshapes
    tscr_wq = nc.dram_tensor("tscr_wq", [1, D], F32R)
    tscr_dh = nc.dram_tensor("tscr_dh", [1, D], F32R)
    tscr_cb = nc.dram_tensor("tscr_cb", [1, JPAD], F32)

    ET = D // 128   # 16 e-tiles (contraction over embedding)
    DT = D // 128   # 16 d-tiles

    with tile.TileContext(nc) as tc, ExitStack() as ctx:
        singles = ctx.enter_context(tc.tile_pool(name="singles", bufs=1))
        thp = ctx.enter_context(tc.tile_pool(name="thp", bufs=4))
        relup = ctx.enter_context(tc.tile_pool(name="relup", bufs=3))
        wstream = ctx.enter_context(tc.tile_pool(name="wstream", bufs=2))
        pbcp = ctx.enter_context(tc.tile_pool(name="pbcp", bufs=1))
        scrp = ctx.enter_context(tc.tile_pool(name="scrp", bufs=1))
        rowp = ctx.enter_context(tc.tile_pool(name="rowp", bufs=2))
        psA = ctx.enter_context(tc.tile_pool(name="psA", bufs=2, space="PSUM"))
        psB = ctx.enter_context(tc.tile_pool(name="psB", bufs=1, space="PSUM"))
        psC = ctx.enter_context(tc.tile_pool(name="psC", bufs=2, space="PSUM"))

        # ---- resident constants / accumulators ----
        wctx_res = singles.tile([128, ET, D], BF16, tag="wctx_res")
        enc_res = singles.tile([128, ET, SS], BF16, tag="enc_res")
        th_all = singles.tile([128, DT, SS // 2], BF16, tag="th_all")
        w1u_res = singles.tile([128, DT, JPAD], BF16, tag="w1u_res")
        vcur_sb = singles.tile([128, ET], F32R, tag="vcur_sb")
        vsp_sb = singles.tile([128, ET], F32R, tag="vsp_sb")
        w2_sb = singles.tile([128, 2], BF16, tag="w2_sb")
        ones_sb = singles.tile([1, 128], F32R, tag="ones_sb")
        negrow = singles.tile([1, BLK], F32, tag="negrow")
        tcol_wq = singles.tile([128, ET], F32R, tag="tcol_wq")
        tcol_dh = singles.tile([128, ET], F32R, tag="tcol_dh")
        cb_cols = singles.tile([128, 2], F32, tag="cb_cols")
        cb_row = singles.tile([1, JPAD], F32, tag="cb_row")
        b1_sb = singles.tile([1, JPAD], F32, tag="b1_sb")
        mrow = singles.tile([1, NBLK], F32, tag="mrow")
        lrow = singles.tile([1, NBLK], F32, tag="lrow")
        negm = singles.tile([1, 1], F32, tag="negm")
        wrow = singles.tile([1, NBLK], F32R, tag="wrow")
        st_sb = singles.tile([1, 2], F32, tag="st_sb")
        ctxp = [
            singles.tile([128, NBLK], F32, tag=f"ctxp{e}", name=f"ctxp{e}")
            for e in range(ET)
        ]
        ctxcol = singles.tile([128, 1], F32, tag="ctxcol")

        nc.vector.memset(negrow, NEG)

        # vectors arrive as [D,1]; view as [p, t] with p the partition axis
        nc.sync.dma_start(out=vcur_sb, in_=vcur.ap().rearrange("(t p) o -> p (t o)", p=128))
        nc.sync.dma_start(out=vsp_sb, in_=vsp.ap().rearrange("(t p) o -> p (t o)", p=128))
        nc.sync.dma_start(out=w2_sb, in_=w2T.ap().rearrange("(t p) o -> p (t o)", p=128))
        nc.sync.dma_start(out=ones_sb, in_=ones_d[:, :])
        nc.sync.dma_start(out=b1_sb, in_=b1[:, :])

        # ---- GEMV phase: dh = W_info @ cur, wq = W_q @ s_prev + b_q ----
        for wmatT, vsb, tcol, tscr, with_bias in [
            (wqT, vsp_sb, tcol_wq, tscr_wq, True),
            (winfoT, vcur_sb, tcol_dh, tscr_dh, False),
        ]:
            for dg in range(D // 512):
                ps_row = psA.tile([1, 512], F32, tag="psA", name="ps_row")
                for e in range(ET):
                    wt = wstream.tile([128, 512], F32R, tag="wstream", name="wt")
                    nc.sync.dma_start(
                        out=wt, in_=wmatT[e * 128 : (e + 1) * 128, dg * 512 : (dg + 1) * 512]
                    )
                    nc.tensor.matmul(
                        ps_row, vsb[:, e : e + 1], wt,
                        start=(e == 0), stop=(e == ET - 1),
                    )
                throw = rowp.tile([1, 512], F32R, tag="throw", name="throw")
                if with_bias:
                    bqt = rowp.tile([1, 512], F32, tag="bqt", name="bqt")
                    nc.sync.dma_start(out=bqt, in_=bq[:, dg * 512 : (dg + 1) * 512])
                    nc.vector.tensor_add(throw, ps_row, bqt)
                    nc.scalar.activation(out=throw, in_=throw, func=ACT.Tanh)
                else:
                    nc.scalar.activation(out=throw, in_=ps_row, func=ACT.Tanh)
                nc.sync.dma_start(out=tscr[:, dg * 512 : (dg + 1) * 512], in_=throw)
            # tanh row -> [128, 16] column layout via DRAM roundtrip
            nc.sync.dma_start(
                out=tcol, in_=tscr.ap().rearrange("a (t p) -> p (t a)", p=128)
            )

        # ---- c = W1w @ tanh(wq) + W1d @ tanh(dh)  (+ b1) ----
        ps_c = psC.tile([1, JPAD], F32, tag="psC", name="ps_c")
        n_c_mm = 2 * DT
        i_mm = 0
        for tcol, wmat in [(tcol_wq, w1wT), (tcol_dh, w1dT)]:
            for t in range(DT):
                wt = wstream.tile([128, JPAD], F32R, tag="wstream", name="wtc")
                nc.sync.dma_start(out=wt, in_=wmat[t * 128 : (t + 1) * 128, :])
                nc.tensor.matmul(
                    ps_c, tcol[:, t : t + 1], wt,
                    start=(i_mm == 0), stop=(i_mm == n_c_mm - 1),
                )
                i_mm += 1
        nc.vector.tensor_add(cb_row, ps_c, b1_sb)
        nc.sync.dma_start(out=tscr_cb[:, :], in_=cb_row)
        nc.sync.dma_start(
            out=cb_cols, in_=tscr_cb.ap().rearrange("a (t p) -> p (t a)", p=128)
        )

        # ---- resident weights + encoder states for the big matmuls ----
        wctx_view = wctxT.ap().rearrange("(te p) d -> p te d", p=128)
        for te in range(ET):
            for dq in range(4):
                nc.sync.dma_start(
                    out=wctx_res[:, te, dq * 512 : (dq + 1) * 512],
                    in_=wctx_view[:, te, dq * 512 : (dq + 1) * 512],
                )
        w1u_view = w1uT.ap().rearrange("(td p) j -> p td j", p=128)
        for td in range(DT):
            nc.sync.dma_start(out=w1u_res[:, td, :], in_=w1u_view[:, td, :])
        for e in range(ET):
            nc.sync.dma_start(
                out=enc_res[:, e, :], in_=encT[e * 128 : (e + 1) * 128, :]
            )

        # ---- main: two halves of 4 blocks; weights stay loaded across blocks ----
        HB = 2                  # blocks per half
        HS = HB * BLK           # 1024
        for h2 in range(2):
            # uh = W_ctx.T-contraction, tanh -> th_all  (one ACT per d-tile)
            for td in range(DT):
                ps_u = psA.tile([128, HS], F32, tag="psA", name="ps_u")
                for e in range(ET):
                    for b4 in range(HB):
                        nc.tensor.matmul(
                            ps_u[:, b4 * BLK : (b4 + 1) * BLK],
                            wctx_res[:, e, td * 128 : (td + 1) * 128],
                            enc_res[:, e, h2 * HS + b4 * BLK : h2 * HS + (b4 + 1) * BLK],
                            start=(e == 0), stop=(e == ET - 1),
                        )
                nc.scalar.activation(out=th_all[:, td, :], in_=ps_u, func=ACT.Tanh)
            # h projection, relu (one ACT per jt)
            relu_half = []
            for jt in range(2):
                ps_h = psB.tile([128, HS], F32, tag="psB", name="ps_h")
                for td in range(DT):
                    for b4 in range(HB):
                        nc.tensor.matmul(
                            ps_h[:, b4 * BLK : (b4 + 1) * BLK],
                            w1u_res[:, td, jt * 128 : (jt + 1) * 128],
                            th_all[:, td, b4 * BLK : (b4 + 1) * BLK],
                            start=(td == 0), stop=(td == DT - 1),
                        )
                reluh = relup.tile([128, HS], BF16, tag="reluh", name="reluh")
                nc.scalar.activation(
                    out=reluh, in_=ps_h, func=ACT.Relu,
                    bias=cb_cols[:, jt : jt + 1], scale=1.0,
                )
                relu_half.append(reluh)
            # scores, block softmax stats, ctx partials
            for b4 in range(HB):
                b = h2 * HB + b4
                ps_s = psC.tile([1, BLK], F32, tag="psC", name="ps_s")
                for jt in range(2):
                    nc.tensor.matmul(
                        ps_s, w2_sb[:, jt : jt + 1],
                        relu_half[jt][:, b4 * BLK : (b4 + 1) * BLK],
                        start=(jt == 0), stop=(jt == 1),
                    )
                mkt = rowp.tile([1, BLK], I32, tag="mkt", name="mkt")
                nc.sync.dma_start(out=mkt, in_=maski[:, b * BLK : (b + 1) * BLK])
                srow = rowp.tile([1, BLK], F32, tag="srow", name="srow")
                nc.vector.select(srow, mkt, negrow, ps_s)
                nc.sync.dma_start(out=oscores[:, b * BLK : (b + 1) * BLK], in_=srow)
                nc.vector.tensor_reduce(
                    out=mrow[:, b : b + 1], in_=srow, axis=AX.X, op=ALU.max
                )
                nc.vector.tensor_scalar_mul(negm, mrow[:, b : b + 1], -1.0)
                prow = rowp.tile([1, BLK], F32R, tag="prow", name="prow")
                nc.scalar.activation(
                    out=prow, in_=srow, func=ACT.Exp,
                    bias=negm[0:1, 0:1], scale=1.0, accum_out=lrow[:, b : b + 1],
                )
                ps_bc = psC.tile([128, BLK], F32, tag="psC", name="ps_bc")
                nc.tensor.matmul(ps_bc, ones_sb, prow, start=True, stop=True)
                pbc = pbcp.tile([128, BLK], F32, tag="pbc", name="pbc")
                nc.vector.tensor_copy(pbc, ps_bc)
                for e in range(ET):
                    scr = scrp.tile([128, BLK], F32, tag="scr", name="scr")
                    nc.vector.tensor_mul(
                        scr, enc_res[:, e, b * BLK : (b + 1) * BLK], pbc
                    )
                    nc.vector.tensor_reduce(
                        out=ctxp[e][:, b : b + 1], in_=scr, axis=AX.X, op=ALU.add
                    )

        # ---- per-core combine ----
        nc.vector.tensor_reduce(out=st_sb[:, 0:1], in_=mrow, axis=AX.X, op=ALU.max)
        nc.vector.tensor_scalar_mul(negm, st_sb[:, 0:1], -1.0)
        nc.scalar.activation(
            out=wrow, in_=mrow, func=ACT.Exp, bias=negm[0:1, 0:1], scale=1.0
        )
        ljunk = rowp.tile([1, NBLK], F32, tag="ljunk", name="ljunk")
        nc.vector.tensor_mul(ljunk, lrow, wrow.bitcast(F32))
        nc.vector.tensor_reduce(
            out=st_sb[:, 1:2], in_=ljunk, axis=AX.X, op=ALU.add
        )
        nc.sync.dma_start(out=ostats[:, :], in_=st_sb)

        ps_wb = psC.tile([128, NBLK], F32, tag="psC", name="ps_wb")
        nc.tensor.matmul(ps_wb, ones_sb, wrow, start=True, stop=True)
        wbc = pbcp.tile([128, NBLK], F32, tag="pbc", name="wbc")
        nc.vector.tensor_copy(wbc, ps_wb)
        for e in range(ET):
            scr8 = scrp.tile([128, NBLK], F32, tag="scr", name="scr8")
            nc.vector.tensor_mul(scr8, ctxp[e], wbc)
            nc.vector.tensor_reduce(
                out=ctxcol[:, 0:1], in_=scr8, axis=AX.X, op=ALU.add
            )
            nc.sync.dma_start(out=octx[e * 128 : (e + 1) * 128, 0:1], in_=ctxcol)

    nc.compile()
    return nc


_CACHE = {}


def _marshal(s_prev, enc_hs, cur_men_rep, src_mask, W_info, W_ctx, W_q, b_q,
             W_p1, b_p1, W_p2, b_p2):
    f = np.float32
    import ml_dtypes
    bf = ml_dtypes.bfloat16
    encT = np.ascontiguousarray(np.asarray(enc_hs, f)[0].T).astype(bf)  # [D, S]
    shared = {
        "wctxT": np.ascontiguousarray(np.asarray(W_ctx, f).T).astype(bf),
        "winfoT": np.ascontiguousarray(np.asarray(W_info, f).T),
        "wqT": np.ascontiguousarray(np.asarray(W_q, f).T),
        "vcur": np.asarray(cur_men_rep, f).reshape(D, 1).copy(),
        "vsp": np.asarray(s_prev, f).reshape(D, 1).copy(),
        "bq": np.asarray(b_q, f).reshape(1, D).copy(),
        "ones": np.ones((1, 128), f),
    }
    W1 = np.asarray(W_p1, f)
    for name, sl in [("w1uT", slice(0, D)), ("w1wT", slice(D, 2 * D)),
                     ("w1dT", slice(2 * D, 3 * D))]:
        t = np.zeros((D, JPAD), f)
        t[:, :H] = W1[:, sl].T
        shared[name] = t.astype(bf) if name == "w1uT" else t
    w2 = np.zeros((JPAD, 1), f)
    w2[:H, 0] = np.asarray(W_p2, f)[0]
    shared["w2T"] = w2.astype(bf)
    b1pad = np.zeros((1, JPAD), f)
    b1pad[0, :H] = np.asarray(b_p1, f)
    shared["b1"] = b1pad
    mk = np.ascontiguousarray(np.asarray(src_mask).reshape(S).astype(np.int32))
    in_maps = []
    for c in range(NCORES):
        m = dict(shared)
        m["encT"] = np.ascontiguousarray(encT[:, c * SS : (c + 1) * SS])
        m["maski"] = mk[c * SS : (c + 1) * SS].reshape(1, SS).copy()
        in_maps.append(m)
    return in_maps


def kernel(**inputs):
    if "nc" not in _CACHE:
        _CACHE["nc"] = build()
    nc = _CACHE["nc"]
    in_maps = _marshal(**inputs)
    res = run_bass_kernel_spmd(nc, in_maps, list(range(NCORES)))
    scores = np.concatenate([res.results[c]["oscores"][0] for c in range(NCORES)])
    m_core = np.array([res.results[c]["ostats"][0, 0] for c in range(NCORES)])
    l_core = np.array([res.results[c]["ostats"][0, 1] for c in range(NCORES)])
    ctx_core = np.stack([res.results[c]["octx"][:, 0] for c in range(NCORES)])
    mg = m_core.max()
    wc = np.exp(m_core - mg)
    Z = float((wc * l_core).sum())
    attn = (np.exp(scores - mg) / Z).astype(np.float32)
    ctx = ((wc[:, None] * ctx_core).sum(0, keepdims=True) / Z).astype(np.float32)
    return attn, ctx
